# revision 1
# baseline (speedup 1.0000x reference)
"""Trainium2 Bass kernel for nn_Decoder_F_12120397709391 (retrieval_knn).

out = mlp(emb) + knn_interpolate(l_y, l_pos, h_pos)   (K=3, inverse-d2 weights)

Strategy (8 cores, data-parallel over N_h):
  - Each core gets 4096 fine points; l_pos / l_y / MLP weights replicated.
  - Stage 1 (TensorE): approximate score s = 2*h'.l' - |l'|^2 (coords shifted
    by -0.5 for accuracy), via a K=8 matmul against a precomputed [8, 8192]
    table. PSUM chunks are group-min-reduced (G=8) on VectorE -> [128, 1024]
    group maxes; top-8 groups via max8/max_index => 64 candidates/point.
  - Stage 2 (exact): gather the 64 candidates' original coords with an
    indirect DMA, recompute d2 = (hx-lx)^2+(hy-ly)^2+(hz-lz)^2 elementwise in
    fp32 (bitwise-matching the reference), and select top-3 with
    lowest-index tie-breaking.
  - Gather the 3 l_y rows per point by indirect DMA; weighted sum with
    w = 1/max(d2,1e-16), normalized.
  - MLP on TensorE with transposed activations (per-partition bias+relu
    fused into the ScalarE PSUM eviction), final transpose + add interp.
"""

import numpy as np

import concourse.bacc as bacc
import concourse.bass as bass
import concourse.mybir as mybir
from concourse.bass import ds, ts
from concourse.bass_utils import run_bass_kernel_spmd
from concourse.masks import make_identity
from concourse.tile import TileContext

FP = mybir.dt.float32
FR = mybir.dt.float32r
U32 = mybir.dt.uint32
I32 = mybir.dt.int32
BF = mybir.dt.bfloat16

N_DEV = 8
N_H, N_L, H, O = 32768, 8192, 512, 128
NH_D = N_H // N_DEV          # 4096 fine points per core
P = 128                      # partitions / tile rows
G = 8                        # coarse group size
NG = N_L // G                # 1024 groups
TOPG = 6                     # groups kept per point
NC_CHUNK = 512               # score chunk (psum bank)
NCHUNK = N_L // NC_CHUNK     # 16

AX = mybir.AxisListType
OP = mybir.AluOpType
AF = mybir.ActivationFunctionType


def build_nc(nh_d=NH_D, finalize=True, knn=True, mlp=True):
    tiles = nh_d // P
    assert tiles % 2 == 0 or tiles == 1
    nc = bacc.Bacc()

    emb = nc.declare_dram_parameter("emb", [nh_d, H], FP, isOutput=False)
    hp4 = nc.declare_dram_parameter("hp4", [nh_d, 4], FP, isOutput=False)
    hp4t = nc.declare_dram_parameter("hp4t", [4, nh_d], FP, isOutput=False)
    lp4 = nc.declare_dram_parameter("lp4", [N_L, 4], FP, isOutput=False)
    lp3t = nc.declare_dram_parameter("lp3t", [3, N_L], FP, isOutput=False)
    ly = nc.declare_dram_parameter("ly", [N_L, O], FP, isOutput=False)
    w1 = nc.declare_dram_parameter("w1", [H, H], FP, isOutput=False)
    w2 = nc.declare_dram_parameter("w2", [H, H], FP, isOutput=False)
    w3 = nc.declare_dram_parameter("w3", [H, O], FP, isOutput=False)
    b1 = nc.declare_dram_parameter("b1", [P, 4], FP, isOutput=False)
    b2 = nc.declare_dram_parameter("b2", [P, 4], FP, isOutput=False)
    b3 = nc.declare_dram_parameter("b3", [P, 1], FP, isOutput=False)
    out = nc.declare_dram_parameter("out", [nh_d, O], FP, isOutput=True)

    with TileContext(nc) as tc:
        with (
            tc.tile_pool(name="const", bufs=1) as cpool,
            tc.tile_pool(name="data", bufs=2) as dpool,
            tc.tile_pool(name="small", bufs=2) as spool,
            tc.tile_pool(name="ps_score", bufs=2, space="PSUM") as pscore,
            tc.tile_pool(name="ps_mm", bufs=2, space="PSUM") as psmm,
            tc.tile_pool(name="ps_t4", bufs=1, space="PSUM") as pst4,
            tc.tile_pool(name="stage1", bufs=1) as s1pool,
        ):
            # ---------------- one-time prep ----------------
            ident = cpool.tile([P, P], FP)
            make_identity(nc, ident[:])

            w1s = cpool.tile([P, 4, H], FP)
            nc.sync.dma_start(out=w1s[:], in_=w1[:].rearrange("(a p) o -> p a o", p=P))
            w2s = cpool.tile([P, 4, H], FP)
            nc.sync.dma_start(out=w2s[:], in_=w2[:].rearrange("(a p) o -> p a o", p=P))
            w3s = cpool.tile([P, 4, O], FP)
            nc.sync.dma_start(out=w3s[:], in_=w3[:].rearrange("(a p) o -> p a o", p=P))
            b1s = cpool.tile([P, 4], FP)
            nc.sync.dma_start(out=b1s[:], in_=b1[:])
            b2s = cpool.tile([P, 4], FP)
            nc.sync.dma_start(out=b2s[:], in_=b2[:])
            b3s = cpool.tile([P, 1], FP)
            nc.sync.dma_start(out=b3s[:], in_=b3[:])
            # fp32r-rounded weight copies (also serve as the DVE const touch
            # so matmul deps collapse to a single DVE wait)
            w1r = cpool.tile([P, 4, H], FR)
            nc.vector.tensor_copy(out=w1r[:], in_=w1s[:])
            w2r = cpool.tile([P, 4, H], FR)
            nc.vector.tensor_copy(out=w2r[:], in_=w2s[:])
            w3r = cpool.tile([P, 4, O], FR)
            nc.vector.tensor_copy(out=w3r[:], in_=w3s[:])
            for cst in (b1s, b2s, b3s, ident):
                nc.vector.tensor_copy(out=cst[:], in_=cst[:])

            # score table [8, N_L]: rows 0-2 = l-0.5 ; rows 3-5 = (l-0.5)^2 ;
            # rows 6-7 = 0.  (compute-engine APs must start at partition 0,
            # so squares are computed in a scratch tile and DMA'd into place)
            l8r = cpool.tile([8, N_L], FR)
            with tc.tile_pool(name="prep", bufs=1) as ppool:
                HN = N_L // 8
                for hh in range(8):
                    sl = ds(hh * HN, HN)
                    l8 = ppool.tile([8, HN], FP, tag="l8")
                    nc.vector.memset(l8[:], 0.0)
                    nc.sync.dma_start(out=l8[0:3, :], in_=lp3t[:, sl])
                    nc.vector.tensor_scalar(
                        out=l8[0:3, :], in0=l8[0:3, :], scalar1=0.5, scalar2=None,
                        op0=OP.subtract,
                    )
                    sq3 = ppool.tile([3, HN], FP, tag="sq3")
                    nc.vector.tensor_mul(out=sq3[:], in0=l8[0:3, :], in1=l8[0:3, :])
                    nc.sync.dma_start(out=l8[3:6, :], in_=sq3[:])
                    nc.vector.tensor_copy(out=l8r[:, sl], in_=l8[:])

            # iota 0..7 as f32 (slot offset within group)
            jt_i = cpool.tile([P, G], I32)
            nc.gpsimd.iota(jt_i[:], pattern=[[1, G]], base=0, channel_multiplier=0)
            jf = cpool.tile([P, G], FP)
            nc.vector.tensor_copy(out=jf[:], in_=jt_i[:])

            # small bias constants for ScalarE activations
            cm1 = cpool.tile([P, 1], FP)
            nc.vector.memset(cm1[:], -1.0)
            cmh = cpool.tile([P, 1], FP)
            nc.vector.memset(cmh[:], -0.5)
            cz = cpool.tile([P, 1], FP)
            nc.vector.memset(cz[:], 0.0)

            # warm up ACT + PE view of the DVE clock so per-tile ops carry a
            # single sync wait
            actw = cpool.tile([1, 1], FP)
            nc.scalar.copy(out=actw[:], in_=ident[0:1, 0:1])
            pew = pst4.tile([P, P], FP)
            nc.tensor.transpose(pew[:], ident[:], ident[:])

            # grouped view of the padded coord table: [NG, G*4]
            lp_grp = lp4[:].rearrange("(g a) d -> g (a d)", a=G)

            def knn_pair(i, ets, h4p, h4ts):
                """kNN interp for a pair of 128-point tiles -> accp [P, 2, O].

                Stage 1 (scoring/tree/top-groups) runs per tile; everything
                from the coord gather on is batched across the pair, with
                per-tile scalars carried as [P, 2] tensors broadcast via
                zero-stride APs.
                """
                NCD = TOPG * G  # 48
                g8is = []
                for j in range(2):
                    h4t = h4ts[j]
                    # lhsT8 = [2h-1 (3 rows); -1 (5 rows)] in fp32r
                    lhsT8 = spool.tile([8, P], FR, tag=f"lhsT8_{j}")
                    nc.vector.memset(lhsT8[:].bitcast(FP), -1.0)
                    nc.scalar.activation(
                        out=lhsT8[0:3, :], in_=h4t[0:3, :], func=AF.Identity,
                        bias=cm1[0:3, :], scale=2.0,
                    )
                    # -(|h'|^2)
                    hs = spool.tile([P, 3], FP, tag=f"hs{j}")
                    h2p = spool.tile([P, 1], FP, tag=f"h2p{j}")
                    nc.scalar.activation(
                        out=hs[:], in_=h4p[:, j, 0:3], func=AF.Square,
                        bias=cmh[:], scale=1.0, accum_out=h2p[:],
                    )
                    h2n = spool.tile([P, 1], FP, tag=f"h2n{j}")
                    nc.scalar.activation(
                        out=h2n[:], in_=h2p[:], func=AF.Identity, bias=cz[:],
                        scale=-1.0,
                    )

                    scb = s1pool.tile([P, N_L], BF, tag=f"scb{j}")
                    t4 = s1pool.tile([P, NG, 4], BF, tag=f"t4_{j}")
                    scv = scb[:].rearrange("p (g e) -> p g e", e=G)
                    NGC = NG // (NCHUNK // 2)
                    for c in range(NCHUNK // 2):
                        ps = pscore.tile([P, 2 * NC_CHUNK], FP, tag="ps_score")
                        for h in range(2):
                            nc.tensor.matmul(
                                ps[:, ds(h * NC_CHUNK, NC_CHUNK)], lhsT=lhsT8[:],
                                rhs=l8r[:, ds((2 * c + h) * NC_CHUNK, NC_CHUNK)],
                                start=True, stop=True,
                            )
                        nc.scalar.activation(
                            out=scb[:, ds(c * 2 * NC_CHUNK, 2 * NC_CHUNK)],
                            in_=ps[:], func=AF.Identity, bias=h2n[:], scale=1.0,
                        )
                        gsl = ds(c * NGC, NGC)
                        nc.vector.tensor_tensor(
                            out=t4[:, gsl, :], in0=scv[:, gsl, 0:4],
                            in1=scv[:, gsl, 4:8], op=OP.max,
                        )
                    t2 = s1pool.tile([P, NG, 2], BF, tag=f"t2_{j}")
                    nc.vector.tensor_tensor(
                        out=t2[:], in0=t4[:, :, 0:2], in1=t4[:, :, 2:4], op=OP.max
                    )
                    gmax = s1pool.tile([P, NG], BF, tag=f"gmax{j}")
                    nc.vector.tensor_tensor(
                        out=gmax[:], in0=t2[:, :, 0], in1=t2[:, :, 1], op=OP.max
                    )
                    g8v = spool.tile([P, 8], BF, tag=f"g8v{j}")
                    nc.vector.max(out=g8v[:], in_=gmax[:])
                    g8i = spool.tile([P, 8], U32, tag=f"g8i{j}")
                    nc.vector.max_index(out=g8i[:], in_max=g8v[:], in_values=gmax[:])
                    g8is.append(g8i)

                # gather candidate coords for the pair: [P, 2, TOPG, G*4]
                cposp = dpool.tile([P, 2, TOPG, G * 4], FP, tag="cposp")
                for j in range(2):
                    for g in range(TOPG):
                        nc.gpsimd.indirect_dma_start(
                            out=cposp[:, j, g, :], out_offset=None,
                            in_=lp_grp,
                            in_offset=bass.IndirectOffsetOnAxis(
                                ap=g8is[j][:, g : g + 1], axis=0
                            ),
                        )

                # exact d2 for all candidates of the pair
                dd = dpool.tile([P, 2, NCD, 4], FP, tag="dd")
                nc.vector.tensor_sub(
                    out=dd[:],
                    in0=cposp[:].rearrange("p j g (a d) -> p j (g a) d", d=4),
                    in1=h4p[:].rearrange("p j d -> p j () d").to_broadcast(
                        [P, 2, NCD, 4]
                    ),
                )
                nc.vector.tensor_mul(out=dd[:], in0=dd[:], in1=dd[:])
                d2 = spool.tile([P, 2, NCD], FP, tag="d2")
                nc.vector.tensor_reduce(out=d2[:], in_=dd[:], op=OP.add, axis=AX.X)

                # global candidate indices as f32: ci = g*8 + j
                g8fp = spool.tile([P, 2, 8], FP, tag="g8fp")
                for j in range(2):
                    nc.scalar.activation(
                        out=g8fp[:, j, :], in_=g8is[j][:], func=AF.Identity,
                        bias=cz[:], scale=float(G),
                    )
                cif = spool.tile([P, 2, TOPG, G], FP, tag="cif")
                nc.vector.tensor_add(
                    out=cif[:],
                    in0=g8fp[:, :, 0:TOPG].rearrange(
                        "p j g -> p j g ()"
                    ).to_broadcast([P, 2, TOPG, G]),
                    in1=jf[:].rearrange("p e -> p () () e").to_broadcast(
                        [P, 2, TOPG, G]
                    ),
                )
                cifl = cif[:].rearrange("p j a b -> p j (a b)")

                def bcast(x2):
                    return x2.rearrange("p j -> p j ()").to_broadcast([P, 2, NCD])

                # ---- exact top-3 selection with lowest-index tie-break ----
                nd = spool.tile([P, 2, NCD], FP, tag="nd")
                nc.vector.tensor_scalar(
                    out=nd[:], in0=d2[:], scalar1=-1.0, scalar2=None, op0=OP.mult
                )
                m8p = spool.tile([P, 2, 8], FP, tag="m8p")
                for j in range(2):
                    nc.vector.max(out=m8p[:, j, :], in_=nd[:, j, :])
                t3v = m8p[:, :, 2]

                m_gt = spool.tile([P, 2, NCD], FP, tag="m_gt")
                nc.vector.tensor_tensor(
                    out=m_gt[:], in0=nd[:], in1=bcast(t3v), op=OP.is_gt
                )
                m_eq = spool.tile([P, 2, NCD], FP, tag="m_eq")
                nc.vector.tensor_tensor(
                    out=m_eq[:], in0=nd[:], in1=bcast(t3v), op=OP.is_equal
                )
                cnt_gt = spool.tile([P, 2], FP, tag="cnt_gt")
                nc.vector.tensor_reduce(
                    out=cnt_gt[:], in_=m_gt[:], op=OP.add, axis=AX.X
                )

                big = spool.tile([P, 2, NCD], FP, tag="big")
                nc.vector.tensor_scalar(
                    out=big[:], in0=m_eq[:], scalar1=-1e9, scalar2=1e9,
                    op0=OP.mult, op1=OP.add,
                )
                tmp = spool.tile([P, 2, NCD], FP, tag="tmp")
                nc.vector.tensor_mul(out=tmp[:], in0=cifl, in1=m_eq[:])
                nc.vector.tensor_add(out=big[:], in0=big[:], in1=tmp[:])

                c1 = spool.tile([P, 2], FP, tag="c1")
                nc.vector.tensor_reduce(out=c1[:], in_=big[:], op=OP.min, axis=AX.X)
                m1 = spool.tile([P, 2, NCD], FP, tag="m1")
                nc.vector.tensor_tensor(
                    out=m1[:], in0=big[:], in1=bcast(c1[:]), op=OP.is_equal
                )
                nc.vector.tensor_scalar(
                    out=tmp[:], in0=m1[:], scalar1=1e9, scalar2=None, op0=OP.mult
                )
                nc.vector.tensor_add(out=big[:], in0=big[:], in1=tmp[:])
                c2 = spool.tile([P, 2], FP, tag="c2")
                nc.vector.tensor_reduce(out=c2[:], in_=big[:], op=OP.min, axis=AX.X)
                m2 = spool.tile([P, 2, NCD], FP, tag="m2")
                nc.vector.tensor_tensor(
                    out=m2[:], in0=big[:], in1=bcast(c2[:]), op=OP.is_equal
                )
                f2 = spool.tile([P, 2], FP, tag="f2")
                nc.vector.tensor_scalar(
                    out=f2[:], in0=cnt_gt[:], scalar1=1.5, scalar2=None, op0=OP.is_le
                )
                nc.vector.tensor_tensor(
                    out=m2[:], in0=m2[:], in1=bcast(f2[:]), op=OP.mult
                )
                mask = spool.tile([P, 2, NCD], FP, tag="mask")
                nc.vector.tensor_add(out=mask[:], in0=m_gt[:], in1=m1[:])
                nc.vector.tensor_add(out=mask[:], in0=mask[:], in1=m2[:])

                # ---- extract the 3 selected (ascending global index) ----
                nc.vector.tensor_scalar(
                    out=big[:], in0=mask[:], scalar1=-1e9, scalar2=1e9,
                    op0=OP.mult, op1=OP.add,
                )
                nc.vector.tensor_mul(out=tmp[:], in0=cifl, in1=mask[:])
                nc.vector.tensor_add(out=big[:], in0=big[:], in1=tmp[:])

                w64 = spool.tile([P, 2, NCD], FP, tag="w64")
                nc.vector.tensor_scalar(
                    out=w64[:], in0=d2[:], scalar1=1e-16, scalar2=None, op0=OP.max
                )
                nc.vector.reciprocal(out=w64[:], in_=w64[:])

                eks, wks = [], []
                scr = spool.tile([P, 2, NCD], FP, tag="scr")
                for k in range(3):
                    ek = spool.tile([P, 2], FP, tag=f"ek{k}")
                    nc.vector.tensor_reduce(
                        out=ek[:], in_=big[:], op=OP.min, axis=AX.X
                    )
                    mk = spool.tile([P, 2, NCD], FP, tag=f"mk{k}")
                    nc.vector.tensor_tensor(
                        out=mk[:], in0=big[:], in1=bcast(ek[:]), op=OP.is_equal
                    )
                    if k < 2:
                        nc.vector.tensor_scalar(
                            out=scr[:], in0=mk[:], scalar1=1e9, scalar2=None,
                            op0=OP.mult,
                        )
                        nc.vector.tensor_add(out=big[:], in0=big[:], in1=scr[:])
                    wk = spool.tile([P, 2], FP, tag=f"wk{k}")
                    nc.vector.tensor_mul(out=scr[:], in0=mk[:], in1=w64[:])
                    nc.vector.tensor_reduce(
                        out=wk[:], in_=scr[:], op=OP.max, axis=AX.X
                    )
                    eks.append(ek)
                    wks.append(wk)

                # normalize weights: wn_k = w_k / (w0+w1+w2)
                ssum = spool.tile([P, 2], FP, tag="ssum")
                nc.vector.tensor_add(out=ssum[:], in0=wks[0][:], in1=wks[1][:])
                nc.vector.tensor_add(out=ssum[:], in0=ssum[:], in1=wks[2][:])
                rs = spool.tile([P, 2], FP, tag="rs")
                nc.vector.reciprocal(out=rs[:], in_=ssum[:])
                for k in range(3):
                    nc.vector.tensor_mul(out=wks[k][:], in0=wks[k][:], in1=rs[:])

                # indices -> int32 (clamped) and gather l_y rows per (tile, k)
                accp = dpool.tile([P, 2, O], FP, tag="accp")
                ytp = dpool.tile([P, 2, O], FP, tag="ytp")
                for k in range(3):
                    nc.vector.tensor_scalar(
                        out=eks[k][:], in0=eks[k][:], scalar1=float(N_L - 1),
                        scalar2=None, op0=OP.min,
                    )
                    yk = dpool.tile([P, 2, O], FP, tag=f"ykp{k}")
                    for j in range(2):
                        eki = spool.tile([P, 1], I32, tag=f"eki{k}_{j}")
                        nc.scalar.copy(out=eki[:], in_=eks[k][:, j : j + 1])
                        nc.gpsimd.indirect_dma_start(
                            out=yk[:, j, :], out_offset=None,
                            in_=ly[:],
                            in_offset=bass.IndirectOffsetOnAxis(ap=eki[:], axis=0),
                        )
                        dst = accp if k == 0 else ytp
                        nc.scalar.activation(
                            out=dst[:, j, :], in_=yk[:, j, :], func=AF.Identity,
                            scale=wks[k][:, j : j + 1],
                        )
                    if k > 0:
                        nc.vector.tensor_add(out=accp[:], in0=accp[:], in1=ytp[:])
                return accp

            # ---------------- per-pair loop ----------------
            assert tiles % 2 == 0
            npair = tiles // 2
            for i in range(npair):
                ta, tb = 2 * i, 2 * i + 1
                ets, h4ts = [], []
                for t in (ta, tb):
                    et = dpool.tile([P, H], FP, tag=f"et{t % 2}")
                    nc.sync.dma_start(out=et[:], in_=emb[ts(t, P), :])
                    h4t = spool.tile([4, P], FP, tag=f"h4t{t % 2}")
                    nc.sync.dma_start(out=h4t[:], in_=hp4t[:, ts(t, P)])
                    ets.append(et)
                    h4ts.append(h4t)
                h4p = dpool.tile([P, 2, 4], FP, tag="h4p")
                nc.sync.dma_start(
                    out=h4p[:],
                    in_=hp4[ts(i, 2 * P), :].rearrange("(j p) d -> p j d", p=P),
                )

                if knn:
                    accp = knn_pair(i, ets, h4p, h4ts)
                else:
                    accp = dpool.tile([P, 2, O], FP, tag="accp")
                    nc.vector.memset(accp[:], 0.0)

                nsub = 2
                W = nsub * P  # 256
                if mlp:
                    # transposed emb for the pair: eT2 [P, 4, W] fp32r
                    eT2 = dpool.tile([P, 4, W], FR, tag="eT2")
                    for j in range(nsub):
                        for k in range(4):
                            pt = psmm.tile([P, P], FP, tag="ps_mm")
                            nc.tensor.transpose(
                                pt[:], ets[j][:, ds(k * P, P)], ident[:]
                            )
                            nc.scalar.copy(
                                out=eT2[:, k, ds(j * P, P)], in_=pt[:]
                            )

                    x1 = dpool.tile([P, 4, W], FR, tag="x1")
                    for m in range(4):
                        p1 = psmm.tile([P, W], FP, tag="ps_mm")
                        for k in range(4):
                            nc.tensor.matmul(
                                p1[:], lhsT=w1r[:, k, ds(m * P, P)],
                                rhs=eT2[:, k, :],
                                start=(k == 0), stop=(k == 3),
                            )
                        nc.scalar.activation(
                            out=x1[:, m, :], in_=p1[:], func=AF.Relu,
                            bias=b1s[:, m : m + 1], scale=1.0,
                        )

                    x2 = dpool.tile([P, 4, W], FR, tag="x2")
                    for m in range(4):
                        p2 = psmm.tile([P, W], FP, tag="ps_mm")
                        for k in range(4):
                            nc.tensor.matmul(
                                p2[:], lhsT=w2r[:, k, ds(m * P, P)],
                                rhs=x1[:, k, :],
                                start=(k == 0), stop=(k == 3),
                            )
                        nc.scalar.activation(
                            out=x2[:, m, :], in_=p2[:], func=AF.Relu,
                            bias=b2s[:, m : m + 1], scale=1.0,
                        )

                    p3 = psmm.tile([P, W], FP, tag="ps_mm")
                    for k in range(4):
                        nc.tensor.matmul(
                            p3[:], lhsT=w3r[:, k, :], rhs=x2[:, k, :],
                            start=(k == 0), stop=(k == 3),
                        )
                    x3t = dpool.tile([P, W], FP, tag="x3t")
                    nc.scalar.activation(
                        out=x3t[:], in_=p3[:], func=AF.Identity,
                        bias=b3s[:, 0:1], scale=1.0,
                    )

                for j, t in enumerate((ta, tb)):
                    ot = dpool.tile([P, O], FP, tag=f"ot{j}")
                    if mlp:
                        pf = psmm.tile([P, P], FP, tag="ps_mm")
                        nc.tensor.transpose(
                            pf[:], x3t[:, ds(j * P, P)], ident[:]
                        )
                        nc.vector.tensor_add(
                            out=ot[:], in0=pf[:], in1=accp[:, j, :]
                        )
                    else:
                        nc.vector.tensor_copy(out=ot[:], in_=accp[:, j, :])
                    nc.sync.dma_start(out=out[ts(t, P), :], in_=ot[:])

    if finalize:
        nc.finalize()
    return nc


_NC_CACHE = {}


def _get_nc(nh_d=NH_D):
    if nh_d not in _NC_CACHE:
        _NC_CACHE[nh_d] = build_nc(nh_d)
    return _NC_CACHE[nh_d]


def _marshal(emb, l_y, l_pos, h_pos, W1, b1, W2, b2, W3, b3, n_dev=N_DEV):
    nh_d = h_pos.shape[0] // n_dev
    f32 = np.float32
    lp4 = np.zeros((N_L, 4), f32)
    lp4[:, :3] = l_pos
    lp3t = np.ascontiguousarray(l_pos.T.astype(f32))
    hp4 = np.zeros((h_pos.shape[0], 4), f32)
    hp4[:, :3] = h_pos
    hp4t_full = np.ascontiguousarray(hp4.T)
    b1m = np.ascontiguousarray(b1.reshape(4, P).T.astype(f32))
    b2m = np.ascontiguousarray(b2.reshape(4, P).T.astype(f32))
    b3m = np.ascontiguousarray(b3.reshape(P, 1).astype(f32))
    ly = np.ascontiguousarray(l_y.astype(f32))
    w1 = np.ascontiguousarray(W1.astype(f32))
    w2 = np.ascontiguousarray(W2.astype(f32))
    w3 = np.ascontiguousarray(W3.astype(f32))
    in_maps = []
    for d in range(n_dev):
        sl = slice(d * nh_d, (d + 1) * nh_d)
        in_maps.append(
            dict(
                emb=np.ascontiguousarray(emb[sl].astype(f32)),
                hp4=np.ascontiguousarray(hp4[sl]),
                hp4t=np.ascontiguousarray(hp4t_full[:, sl]),
                lp4=lp4, lp3t=lp3t, ly=ly,
                w1=w1, w2=w2, w3=w3, b1=b1m, b2=b2m, b3=b3m,
            )
        )
    return in_maps


def kernel(emb, l_y, l_pos, h_pos, W1, b1, W2, b2, W3, b3, trace=False):
    nh_d = h_pos.shape[0] // N_DEV
    nc = _get_nc(nh_d)
    in_maps = _marshal(emb, l_y, l_pos, h_pos, W1, b1, W2, b2, W3, b3)
    res = run_bass_kernel_spmd(nc, in_maps, list(range(N_DEV)), trace=trace)
    out = np.concatenate([res.results[d]["out"] for d in range(N_DEV)], axis=0)
    if trace:
        return out, res
    return out

